# revision 1
# baseline (speedup 1.0000x reference)
"""KNN feature upsampling (PointNet++ style) on 8 Trainium2 NeuronCores.

Problem: for each of B*N query points, find the 3 nearest of M reference
points (squared L2), inverse-distance-weight their C-dim features, and sum.

Sharding: data-parallel — 8 cores = 4 batches x 2 halves of N.

Per-core pipeline, per 128-query tile (engine assignment chosen to balance):
  PE   : s = -(squared distance) [128, M] via a 24-row bf16-split contraction
         (near-fp32 accuracy: each fp32 operand split into 3 bf16 limbs;
         limb products are exact in the fp32 PSUM accumulator).
  ACT  : PSUM->SBUF copy of s; per-partition weight scaling of gathered rows.
  DVE  : max8 + max_index (top-3 of 2048), batched weight math, final add.
  Pool : 3 indirect-DMA feature-row gathers + first add.
  sync : output DMA.
"""

import numpy as np
import ml_dtypes

from concourse import bacc, mybir
from concourse import tile
from concourse.bass import IndirectOffsetOnAxis
from concourse.bass_utils import run_bass_kernel_spmd

B, N, M, C = 4, 16384, 2048, 512
NCORES = 8
SPLITS_PER_BATCH = NCORES // B  # 2
NSH = N // SPLITS_PER_BATCH     # 8192 queries per core
P = 128
NT = NSH // P                   # 64 tiles per core
GRP = 4                         # tiles per weight-math batch
KNN = 3
KROWS = 24                      # contraction rows of the bf16-split distance matmul
EPS = 1e-8

F32 = mybir.dt.float32
BF16 = mybir.dt.bfloat16
U32 = mybir.dt.uint32

_cached = {}


def _build_program(reps=1):
    nc = bacc.Bacc(
        "TRN2",
        target_bir_lowering=False,
        debug=False,
        enable_asserts=False,
        num_devices=NCORES,
        num_swdge_queues=4,
    )
    L = nc.dram_tensor("L", [KROWS, NSH], BF16, kind="ExternalInput")
    R = nc.dram_tensor("R", [KROWS, M], BF16, kind="ExternalInput")
    HF = nc.dram_tensor("HF", [M, C], F32, kind="ExternalInput")
    OUT = nc.dram_tensor("out", [NSH, C], F32, kind="ExternalOutput")

    mult = mybir.AluOpType.mult
    add = mybir.AluOpType.add

    with tile.TileContext(nc) as tc:
        with (
            tc.tile_pool(name="const", bufs=1) as cpool,
            tc.tile_pool(name="pss", bufs=4, space="PSUM") as pss,
            tc.tile_pool(name="sb", bufs=5) as sb,
            tc.tile_pool(name="sbg", bufs=2) as sbg,
        ):
            L_sb = cpool.tile([KROWS, NSH], BF16)
            R_sb = cpool.tile([KROWS, M], BF16)
            nc.sync.dma_start(L_sb[:], L.ap())
            nc.sync.dma_start(R_sb[:], R.ap())

            import contextlib
            rep_ctx = tc.For_i(0, reps, 1) if reps > 1 else contextlib.nullcontext()
            with rep_ctx:
              for grp in range(NT // GRP):
                  v8g = sbg.tile([P, 8 * GRP], F32, tag="v8g")
                  i8g = sbg.tile([P, 8 * GRP], U32, tag="i8g")
                  w3g = sbg.tile([P, KNN * GRP], F32, tag="w3g")

                  for ti in range(GRP):
                      t = grp * GRP + ti
                      # ---- distances: s = 2 q.p - |q|^2 - |p|^2  (= -d) ----
                      s_sb = sb.tile([P, M], F32, tag="s_sb")
                      for h in range(2):  # two PSUM halves of 1024
                          s_ps = pss.tile([P, M // 2], F32, tag="s_ps")
                          for j in range(2):  # 512-wide matmuls
                              nc.tensor.matmul(
                                  s_ps[:, j * 512:(j + 1) * 512],
                                  lhsT=L_sb[:, t * P:(t + 1) * P],
                                  rhs=R_sb[:, (2 * h + j) * 512:(2 * h + j + 1) * 512],
                                  start=True,
                                  stop=True,
                              )
                          nc.scalar.copy(s_sb[:, h * 1024:(h + 1) * 1024], s_ps[:])

                      # ---- top-3 (largest s = smallest d) + indices ----
                      v8 = v8g[:, 8 * ti:8 * ti + 8]
                      i8 = i8g[:, 8 * ti:8 * ti + 8]
                      nc.vector.max(out=v8, in_=s_sb[:])
                      nc.vector.max_index(out=i8, in_max=v8, in_values=s_sb[:])

                  # ---- batched inverse-distance weights for the group ----
                  sel = v8g[:].rearrange("p (t k) -> p t k", k=8)[:, :, 0:KNN]
                  dp = sbg.tile([P, GRP * KNN], F32, tag="dp")
                  dp3 = dp[:].rearrange("p (t k) -> p t k", k=KNN)
                  nc.vector.tensor_scalar(dp3, sel, -1.0, EPS, op0=mult, op1=add)
                  r3 = sbg.tile([P, GRP * KNN], F32, tag="r3")
                  nc.vector.reciprocal(r3[:], dp[:])
                  r33 = r3[:].rearrange("p (t k) -> p t k", k=KNN)
                  rs = sbg.tile([P, GRP], F32, tag="rs")
                  nc.vector.tensor_reduce(rs[:], r33, axis=mybir.AxisListType.X, op=add)
                  rsi = sbg.tile([P, GRP], F32, tag="rsi")
                  nc.vector.reciprocal(rsi[:], rs[:])
                  rsib = rsi[:].rearrange("p (t o) -> p t o", o=1).to_broadcast([P, GRP, KNN])
                  w3g3 = w3g[:].rearrange("p (t k) -> p t k", k=KNN)
                  nc.vector.tensor_tensor(out=w3g3, in0=r33, in1=rsib, op=mult)

                  for ti in range(GRP):
                      t = grp * GRP + ti
                      i8 = i8g[:, 8 * ti:8 * ti + 8]
                      # ---- gather the 3 neighbor feature rows ----
                      g = []
                      for k in range(KNN):
                          gk = sb.tile([P, C], F32, tag=f"g{k}")
                          gi = nc.gpsimd.indirect_dma_start(
                              out=gk[:],
                              out_offset=None,
                              in_=HF.ap(),
                              in_offset=IndirectOffsetOnAxis(ap=i8[:, k:k + 1], axis=0),
                          )
                          gi.ins.queue = f"qPoolDynamic{k or ''}"
                          g.append(gk)
                      # ---- scale by weights (ACT, per-partition scalar) ----
                      sc = []
                      for k in range(KNN):
                          sck = sb.tile([P, C], F32, tag=f"sc{k}")
                          nc.scalar.mul(sck[:], g[k][:], w3g[:, KNN * ti + k:KNN * ti + k + 1])
                          sc.append(sck)
                      # ---- sum the three scaled tiles (GPSIMD + DVE) ----
                      x01 = sb.tile([P, C], F32, tag="x01")
                      nc.gpsimd.tensor_add(x01[:], sc[0][:], sc[1][:])
                      ot = sb.tile([P, C], F32, tag="ot")
                      nc.vector.tensor_add(ot[:], x01[:], sc[2][:])
                      nc.sync.dma_start(OUT.ap()[t * P:(t + 1) * P, :], ot[:])

    nc.compile()
    return nc


def _split3_bf16(x64):
    """Split float64 array into 3 bf16 limbs (x ~= l0+l1+l2 to ~2^-24 rel)."""
    l0 = x64.astype(ml_dtypes.bfloat16)
    r = x64 - l0.astype(np.float64)
    l1 = r.astype(ml_dtypes.bfloat16)
    r = r - l1.astype(np.float64)
    l2 = r.astype(ml_dtypes.bfloat16)
    return l0, l1, l2


def _build_sides(pts64, is_query):
    """24 contraction rows for one side of  s = a.b - |q|^2 - |p|^2.

    Query side (a = 2q):  rows carry a-limbs, |q|^2-limbs, and ones.
    Ref side   (b = p):   rows carry b-limbs, ones, and |p|^2-limbs.
    Row order puts small-magnitude products first to reduce fp32
    accumulation rounding in PSUM.
    """
    n = pts64.shape[0]
    sq = (pts64 ** 2).sum(1)
    one = np.ones((1, n), ml_dtypes.bfloat16)
    if is_query:
        v1, v2, v3 = _split3_bf16(2.0 * pts64.T)       # [3, n] each
        n1, n2, n3 = (x[None] for x in _split3_bf16(-sq))
        rows = [v1, v3, v2, n3, one, n2, one, v1, v2, v1, n1, one]
    else:
        v1, v2, v3 = _split3_bf16(pts64.T)
        n1, n2, n3 = (x[None] for x in _split3_bf16(-sq))
        rows = [v3, v1, v2, one, n3, one, n2, v2, v1, v1, one, n1]
    out = np.concatenate(rows, axis=0)
    assert out.shape[0] == KROWS
    return np.ascontiguousarray(out)


# Row plan (paired q-row x p-row, ordered small products first):
#   0-2  : a1*b3   (~2^-18)     13-15: a1*b2   (~2^-9)
#   3-5  : a3*b1   (~2^-18)     16-18: a2*b1   (~2^-9)
#   6-8  : a2*b2   (~2^-18)     19-21: a1*b1   (O(1))
#   9    : alpha3*1             22   : alpha1*1
#   10   : 1*gamma3             23   : 1*gamma1
#   11   : alpha2*1
#   12   : 1*gamma2
# where a=2q, alpha_i = limbs of -|q|^2, gamma_i = limbs of -|p|^2.


def _selftest_rows():
    rng = np.random.default_rng(0)
    q = rng.random((5, 3))
    p = rng.random((7, 3))
    Lr = _build_sides(q, True).astype(np.float64)
    Rr = _build_sides(p, False).astype(np.float64)
    s = Lr.T @ Rr
    ref = 2 * q @ p.T - (q ** 2).sum(1)[:, None] - (p ** 2).sum(1)[None, :]
    err = np.abs(s - ref).max()
    assert err < 1e-6, err


def _prep_core_inputs(q, hp, hf):
    q64 = q.astype(np.float64)
    p64 = hp.astype(np.float64)
    return {
        "L": _build_sides(q64, True),
        "R": _build_sides(p64, False),
        "HF": np.ascontiguousarray(hf),
    }


def kernel(higher_feats, lower_points, higher_points, _timing=None):
    global _cached
    if "p1" not in _cached:
        _selftest_rows()
        _cached["p1"] = _build_program()
    nc = _cached["p1"]

    in_maps = []
    for c in range(NCORES):
        b, h = divmod(c, SPLITS_PER_BATCH)
        q = lower_points[b, h * NSH:(h + 1) * NSH]
        in_maps.append(_prep_core_inputs(q, higher_points[b], higher_feats[b]))

    res = run_bass_kernel_spmd(nc, in_maps, core_ids=list(range(NCORES)))
    if _timing is not None:
        _timing.append(res)

    out = np.empty((B, N, C), np.float32)
    for c in range(NCORES):
        b, h = divmod(c, SPLITS_PER_BATCH)
        out[b, h * NSH:(h + 1) * NSH] = res.results[c]["out"]
    return out

